# revision 20
# baseline (speedup 1.0000x reference)
"""Cross-modal selective-scan (ASSM) kernel for 8 TRN2 NeuronCores.

Sharding: one core per (batch, stream) pair: core = b*2 + s, s=0 rgb / s=1 e.
Each core computes the full forward for its stream (routing+gumbel of the
OTHER stream feeds C — cross-modal), the L=4096 selective scan over
(D=192, N=16) states, and the output layernorm. Outputs are gathered on host.

v3 highlights:
 - matmuls in bf16 (x-projections / routing / yacc) and fp32r (scan args),
   never plain fp32 on big streams -> ~4x PE throughput per column.
 - the state-injection/readout chain (dBu, h, ym, B, C) runs in bf16: DVE
   2x mode for the elementwise ops, half the SBUF traffic. The compounding
   decay dA stays fp32 (bf16 decay quantization accumulates over the scan).
 - gumbel noise -ln(-ln u) (+ b2 routing bias) precomputed on host.
 - the next chunk's load/routing/projection preamble is emitted in the
   middle of the current chunk's scan phase (software pipelining).
 - GpSimd handles SBUF-only side ops (w-build, softplus add, one-hot eq,
   y^2); it cannot access PSUM, so PSUM readers stay on Vector/Scalar.
"""

import numpy as np
import ml_dtypes

import concourse.bass as bass
import concourse.bacc as bacc
import concourse.mybir as mybir
import concourse.tile as tile
from concourse.bass_utils import run_bass_kernel_spmd

FP = mybir.dt.float32
FPR = mybir.dt.float32r
BF = mybir.dt.bfloat16
OP = mybir.AluOpType
AF = mybir.ActivationFunctionType
F16 = mybir.dt.float16

B, L, DM, N, R, T, H3 = 4, 4096, 192, 16, 12, 64, 64
P = 128
LC = 1024               # chunk along L
NCH = L // LC           # 4
SUB = 512
LEPS = 1e-5
NT = LC // 128          # 8 token tiles per chunk
# GpSimd offload measured SLOWER: its TT is ~2.7us/[128,1024] and it
# contends with DVE for SBUF ports (scans slowed 2326->2968ns). Keep all
# dBu/ym mults on DVE; a few dBu read PSUM directly (saves Act evac time).
DBU_DVE = {2, 8, 14, 20}

# ---- packed-constant layouts: name -> (col offset, rows, cols) ----
def _pack(shapes):
    m, off = {}, 0
    for nm, r, c in shapes:
        m[nm] = (off, r, c)
        off += c
    return m, off

_CB_SHAPES = [
    ("w1T0", 128, 64), ("w1T1", 64, 64), ("w2T", 64, 64), ("PmRep", 64, 128),
    ("xpC0", 128, 128), ("xpC1", 64, 128), ("xpB0", 128, 128),
    ("xpB1", 64, 128), ("Mdt0", 128, 192), ("Mdt1", 64, 192),
    ("S80", 128, 16 * 128), ("S81", 128, 8 * 64),
    ("WdA0", 128, 16 * 128), ("WdA1", 64, 8 * 128),
]
CBMAP, CBTOT = _pack(_CB_SHAPES)

_CR_SHAPES = [
    ("onc0", 128, 1), ("onc1", 64, 1), ("lngr0", 1, 128), ("lngr1", 1, 64),
]
CRMAP, CRTOT = _pack(_CR_SHAPES)

_CF_SHAPES = [
    ("identF", 128, 128), ("b1c", 64, 1), ("dtb0", 128, 1), ("dtb1", 64, 1),
    ("invA", 128, 1), ("Dc0", 128, 1), ("Dc1", 64, 1), ("lnb0", 128, 1),
    ("lnb1", 64, 1),
]
CFMAP, CFTOT = _pack(_CF_SHAPES)


def build_program():
    nc = bacc.Bacc("TRN2", target_bir_lowering=False, debug=False)

    xsT0 = nc.declare_dram_parameter("xsT0", [128, L], BF, isOutput=False)
    xsT1 = nc.declare_dram_parameter("xsT1", [64, L], BF, isOutput=False)
    xoT0 = nc.declare_dram_parameter("xoT0", [128, L], BF, isOutput=False)
    xoT1 = nc.declare_dram_parameter("xoT1", [64, L], BF, isOutput=False)
    gq = nc.declare_dram_parameter("gq", [L // 128, 128, T], FP, isOutput=False)
    cpb = nc.declare_dram_parameter("cpb", [128, CBTOT], BF, isOutput=False)
    cpr = nc.declare_dram_parameter("cpr", [128, CRTOT], FPR, isOutput=False)
    cpf = nc.declare_dram_parameter("cpf", [128, CFTOT], FP, isOutput=False)
    yo0 = nc.declare_dram_parameter("yo0", [128, L], FP, isOutput=True)
    yo1 = nc.declare_dram_parameter("yo1", [64, L], FP, isOutput=True)

    with tile.TileContext(nc) as tc:
        with (
            tc.tile_pool(name="const", bufs=1) as cp,
            tc.tile_pool(name="xin", bufs=2) as xp,
            tc.tile_pool(name="dwp", bufs=2) as dwp,
            tc.tile_pool(name="proj", bufs=2) as pj,
            tc.tile_pool(name="route", bufs=2) as rt,
            tc.tile_pool(name="spool", bufs=2) as sp_,
            tc.tile_pool(name="blk", bufs=3) as bk,
            tc.tile_pool(name="hpool", bufs=3) as hp,
            tc.tile_pool(name="ypool", bufs=1) as yp,
            tc.tile_pool(name="rows", bufs=1) as rw,
            tc.tile_pool(name="persist", bufs=1) as pe_,
            tc.tile_pool(name="ps_scan", bufs=2, space="PSUM") as ps_scan,
            tc.tile_pool(name="ps_pre", bufs=1, space="PSUM") as ps_pre,
            tc.tile_pool(name="ps_y", bufs=1, space="PSUM") as ps_y,
        ):
            cbk = cp.tile([128, CBTOT], BF, tag="cpb")
            nc.sync.dma_start(cbk[:], cpb[:])
            crk = cp.tile([128, CRTOT], FPR, tag="cpr")
            nc.sync.dma_start(crk[:], cpr[:])
            cfk = cp.tile([128, CFTOT], FP, tag="cpf")
            nc.sync.dma_start(cfk[:], cpf[:])

            def cb(name):
                off, r, c = CBMAP[name]
                return cbk[0:r, off:off + c]

            def cr(name):
                off, r, c = CRMAP[name]
                return crk[0:r, off:off + c]

            def cf(name):
                off, r, c = CFMAP[name]
                return cfk[0:r, off:off + c]

            def mm512(out, lhsT, rhs, start, stop):
                # ISA caps the moving operand at 512 elements per matmul
                n = rhs.shape[-1]
                for q in range(0, n, 512):
                    e = min(q + 512, n)
                    nc.tensor.matmul(out[:, q:e], lhsT, rhs[:, q:e],
                                     start=start, stop=stop)

            hlast = pe_.tile([P, 24], FP)
            epsc = pe_.tile([128, 1], FP)
            nc.vector.memset(epsc[:], LEPS)

            # xo stays resident for the whole kernel (routing + C-projection)
            xo0a = pe_.tile([128, L], BF, tag="xo0a")
            nc.sync.dma_start(xo0a[:], xoT0[:])
            xo1a = pe_.tile([64, L], BF, tag="xo1a")
            nc.sync.dma_start(xo1a[:], xoT1[:])
            hgk = {}

            def emit_gelu(kc):
                # routing hidden layer for chunk kc (gelu act table)
                ls = kc * LC
                zt = ps_pre.tile([128, LC], FP, tag="pre", name=f"zt{kc}")
                mm512(zt[0:H3, :], cb("w1T0"), xo0a[:, ls:ls + LC], True, False)
                mm512(zt[0:H3, :], cb("w1T1"), xo1a[:, ls:ls + LC], False, True)
                hg = rt.tile([H3, LC], BF, tag="hg", bufs=4, name=f"hg_{kc}")
                nc.scalar.activation(hg[:], zt[0:H3, :], AF.Gelu,
                                     bias=cf("b1c"))
                hgk[kc] = hg

            def emit_preamble(kc):
                """Loads + projections + routing + w-build for chunk kc."""
                ls = kc * LC
                c0 = ls // 128
                C = {}
                xs0 = C["xs0"] = xp.tile([128, LC], BF, tag="xs0", name=f"xs0_{kc}")
                xs1 = C["xs1"] = xp.tile([64, LC], BF, tag="xs1", name=f"xs1_{kc}")
                gt = xp.tile([128, NT * T], FP, tag="gt", name=f"gt_{kc}")
                nc.sync.dma_start(xs0[:], xsT0[:, ls:ls + LC])
                nc.sync.dma_start(xs1[:], xsT1[:, ls:ls + LC])
                nc.sync.dma_start(
                    gt[:].rearrange("p (c t) -> p c t", c=NT),
                    gq[c0:c0 + NT].rearrange("c p t -> p c t"))

                # dt -> softplus -> dl ; w = dl*x   (dw = [dl | w])
                dw0 = C["dw0"] = dwp.tile([128, 2 * LC], BF, tag="dw0", name=f"dw0_{kc}")
                dw1 = C["dw1"] = dwp.tile([64, 2 * LC], BF, tag="dw1", name=f"dw1_{kc}")
                dtp0 = ps_pre.tile([128, LC], FP, tag="pre", name=f"dt0{kc}")
                mm512(dtp0[:], cb("Mdt0")[:, 0:128], xs0[:], True, False)
                mm512(dtp0[:], cb("Mdt1")[:, 0:128], xs1[:], False, True)
                # softplus(x) = ln(exp(x) + 1); x = dt + dtb stays < ~3 here.
                # Both Exp ops before both Ln ops: the act tables the
                # compiler picks keep exp and ln apart, so interleaving
                # would cost a 1.3us table swap per switch.
                sp0 = sp_.tile([128, LC], FP, tag="sp0", name=f"sp0_{kc}")
                nc.scalar.activation(sp0[:], dtp0[:], AF.Exp, bias=cf("dtb0"))
                dtp1 = ps_pre.tile([64, LC], FP, tag="pre", name=f"dt1{kc}")
                mm512(dtp1[:], cb("Mdt0")[:, 128:DM], xs0[:], True, False)
                mm512(dtp1[:], cb("Mdt1")[:, 128:DM], xs1[:], False, True)
                sp1 = sp_.tile([64, LC], FP, tag="sp1", name=f"sp1_{kc}")
                nc.scalar.activation(sp1[:], dtp1[:], AF.Exp, bias=cf("dtb1"))
                nc.scalar.activation(dw0[:, 0:LC], sp0[:], AF.Ln, bias=1.0)
                nc.scalar.activation(dw1[:, 0:LC], sp1[:], AF.Ln, bias=1.0)
                nc.gpsimd.tensor_tensor(dw0[:, LC:2 * LC], dw0[:, 0:LC],
                                        xs0[:], OP.mult)
                nc.gpsimd.tensor_tensor(dw1[:, LC:2 * LC], dw1[:, 0:LC],
                                        xs1[:], OP.mult)

                # Brep and Crep live side by side in one [128, 2LC] tile so
                # the per-block dBu/ym multiplies can fuse into a single
                # [128, 2LC] DVE op (halves the per-op overhead)
                bc = C["bc"] = pj.tile([128, 2 * LC], BF, tag="bc", name=f"bc_{kc}")
                bp = ps_pre.tile([128, LC], FP, tag="pre", name=f"bp{kc}")
                mm512(bp[:], cb("xpB0"), xs0[:], True, False)
                mm512(bp[:], cb("xpB1"), xs1[:], False, True)
                nc.scalar.copy(bc[:, 0:LC], bp[:])

                # routing of the other stream -> one-hot OT
                if kc not in hgk:
                    emit_gelu(kc)
                hg = hgk[kc]
                z2 = ps_scan.tile([128, NT * T], FP, tag="scan", name=f"z2{kc}")
                for i in range(NT):
                    nc.tensor.matmul(z2[:, i * T:(i + 1) * T],
                                     hg[:, i * 128:(i + 1) * 128], cb("w2T"),
                                     start=True, stop=True)
                zg = rt.tile([128, NT * T], FP, tag="zg", name=f"zg_{kc}")
                nc.vector.tensor_tensor(zg[:], z2[:], gt[:], OP.add)
                oh = rt.tile([128, NT * T], FP, tag="oh", name=f"oh_{kc}")
                for i in range(NT):
                    sl = slice(i * T, (i + 1) * T)
                    m8 = rt.tile([128, 8], FP, tag="m8", bufs=3, name=f"m8_{kc}_{i}")
                    nc.vector.max(m8[:], zg[:, sl])
                    # is_equal on GpSimd (SBUF-only op; keeps DVE free for scans)
                    nc.gpsimd.tensor_scalar(oh[:, sl], zg[:, sl], m8[:, 0:1],
                                            None, OP.is_equal)
                tp = ps_pre.tile([128, LC], FP, tag="pre", name=f"tp{kc}")
                for i in range(NT):
                    nc.tensor.transpose(tp[0:T, i * 128:(i + 1) * 128],
                                        oh[:, i * T:(i + 1) * T], cf("identF"))
                OT = rt.tile([T, LC], BF, tag="OT", name=f"OT_{kc}")
                nc.scalar.copy(OT[:], tp[0:T, :])

                cpp = ps_pre.tile([128, LC], FP, tag="pre", name=f"cp{kc}")
                mm512(cpp[:], cb("xpC0"), xo0a[:, ls:ls + LC], True, False)
                mm512(cpp[:], cb("xpC1"), xo1a[:, ls:ls + LC], False, False)
                mm512(cpp[:], cb("PmRep"), OT[:], False, True)
                nc.scalar.copy(bc[:, LC:2 * LC], cpp[:])
                C["kc"] = kc
                C["hl_pend"] = []
                C["yacc_pend"] = []
                C["PT"] = {}
                C["pend_ym"] = {}
                return C


            def emit_yacc(C):
                kc = C["kc"]
                j, sc, rows_, first, last = C["yacc_pend"].pop(0)
                ym = C["pend_ym"].pop(j)
                yac = C["yac0"] if j < 16 else C["yac1"]
                mm512(yac[0:rows_, :], sc, ym, first, last)
                if j == 15:
                    yD0 = C["yD0"] = yp.tile([128, LC], FPR, tag="yD0",
                                             name=f"yD0_{kc}")
                    nc.vector.scalar_tensor_tensor(
                        yD0[:], C["xs0"][:], cf("Dc0"), C["yac0"][:],
                        OP.mult, OP.add)
                    C["yac1"] = ps_y.tile([64, LC], FP, tag="y",
                                          name=f"y1_{kc}")

            def emit_block(C, j):
                kc = C["kc"]
                if j < 16:
                    dwt = C["dw0"]
                    wa = cb("WdA0")[:, j * P:(j + 1) * P]
                    sc = cb("S80")[:, j * 128:(j + 1) * 128]
                    rows_ = P
                    first, last = j == 0, j == 15
                else:
                    dwt = C["dw1"]
                    wa = cb("WdA1")[:, (j - 16) * P:(j - 15) * P]
                    sc = cb("S81")[:, (j - 16) * 64:(j - 15) * 64]
                    rows_ = 64
                    first, last = j == 16, j == 23
                # deferred hlast copies (Act) — 2 blocks late so Act never
                # stalls waiting for the scan of the current block
                while C["hl_pend"] and C["hl_pend"][0][0] <= j - 2:
                    _, jj, hh = C["hl_pend"].pop(0)
                    nc.scalar.copy(hlast[:, jj:jj + 1], hh)
                mmpA = ps_scan.tile([128, LC], FP, tag="scan",
                                    name=f"mmA{kc}_{j}")
                mm512(mmpA[:], wa, dwt[:, 0:LC], True, True)
                dA = bk.tile([P, LC], F16, tag="dA", name=f"dA_{kc}_{j}")
                nc.scalar.activation(dA[:], mmpA[:], AF.Exp)
                mmpB = ps_scan.tile([128, LC], FP, tag="scan",
                                    name=f"mmB{kc}_{j}")
                mm512(mmpB[:], wa, dwt[:, LC:2 * LC], True, True)
                bc = C["bc"]
                # paired elementwise: wcp(j) lands next to h(j-1) in the
                # previous block's PT tile, so ONE [128, 2LC] multiply
                # against [Brep|Crep] yields dBu(j) and ym(j-1) together
                if j == 0:
                    wcp = bk.tile([P, LC], BF, tag="wcp", bufs=2, name=f"wcp_{kc}_0")
                    nc.scalar.copy(wcp[:], mmpB[:])
                    dBu0 = bk.tile([P, LC], BF, tag="dBu", bufs=2, name=f"dBu_{kc}_0")
                    nc.vector.tensor_tensor(dBu0[:], wcp[:], bc[:, 0:LC],
                                            OP.mult)
                    data1 = dBu0[:]
                else:
                    PTp = C["PT"].pop(j - 1)
                    nc.scalar.copy(PTp[:, 0:LC], mmpB[:])
                    out2 = hp.tile([P, 2 * LC], BF, tag="out2", bufs=3,
                                   name=f"out2_{kc}_{j}")
                    nc.vector.tensor_tensor(out2[:], PTp[:], bc[:], OP.mult)
                    C["pend_ym"][j - 1] = out2[:, LC:2 * LC]
                    data1 = out2[:, 0:LC]
                PT = hp.tile([P, 2 * LC], BF, tag="PT", bufs=3,
                             name=f"PT_{kc}_{j}")
                C["PT"][j] = PT
                init = 0.0 if kc == 0 else hlast[:, j:j + 1]
                nc.vector.tensor_tensor_scan(PT[:, LC:2 * LC], dA[:], data1,
                                             init, OP.mult, OP.add)
                if kc < NCH - 1:
                    C["hl_pend"].append((j, j, PT[:, 2 * LC - 1:2 * LC]))
                if j == 23:
                    ymL = hp.tile([P, LC], BF, tag="ymL", name=f"ymL_{kc}")
                    nc.vector.tensor_tensor(ymL[:], PT[:, LC:2 * LC],
                                            bc[:, LC:2 * LC], OP.mult)
                    C["pend_ym"][23] = ymL[:]
                # yacc matmuls run late so PE never waits on the DVE scan
                # pipeline mid-stream (HAM throttle avoidance); ym(j) only
                # exists after block j+1's paired multiply
                C["yacc_pend"].append((j, sc, rows_, first, last))
                while len(C["yacc_pend"]) > 2:
                    emit_yacc(C)

            def emit_ln_part1(C):
                """Yacc drain + row sums for chunk C. The DVE-dependent tail
                (emit_ln_part2) is deferred into the next chunk's block loop
                so DVE's FIFO isn't blocked on the PE sum matmuls while the
                next chunk's scans are ready to run."""
                kc = C["kc"]
                while C["yacc_pend"]:
                    emit_yacc(C)
                while C["hl_pend"]:
                    _, jj, hh = C["hl_pend"].pop(0)
                    nc.scalar.copy(hlast[:, jj:jj + 1], hh)
                yD0 = C["yD0"]
                yD1 = C["yD1"] = yp.tile([64, LC], FPR, tag="yD1",
                                         name=f"yD1_{kc}")
                nc.vector.scalar_tensor_tensor(
                    yD1[:], C["xs1"][:], cf("Dc1"), C["yac1"][:],
                    OP.mult, OP.add)
                ysq0 = yp.tile([128, LC], FPR, tag="ysq0", name=f"ysq0_{kc}")
                nc.scalar.activation(ysq0[:], yD0[:].bitcast(FP), AF.Square)
                ysq1 = yp.tile([64, LC], FPR, tag="ysq1", name=f"ysq1_{kc}")
                nc.scalar.activation(ysq1[:], yD1[:].bitcast(FP), AF.Square)

                # sums live in the "pre" ring: the preamble is done with it
                # here, and polluting the "scan" ring would stall the next
                # chunk's mmpA/mmpB allocations behind the LN reads
                s1p = ps_pre.tile([128, LC], FP, tag="pre", name=f"s1{kc}")
                mm512(s1p[0:1, :], cr("onc0"), yD0[:], True, False)
                mm512(s1p[0:1, :], cr("onc1"), yD1[:], False, True)
                s1row = C["s1row"] = rw.tile([1, LC], FP, tag="s1row",
                                             name=f"s1row_{kc}")
                nc.scalar.copy(s1row[:], s1p[0:1, :])
                s2p = ps_pre.tile([128, LC], FP, tag="pre", name=f"s2{kc}")
                mm512(s2p[0:1, :], cr("onc0"), ysq0[:], True, False)
                mm512(s2p[0:1, :], cr("onc1"), ysq1[:], False, True)
                s2row = C["s2row"] = rw.tile([1, LC], FP, tag="s2row",
                                             name=f"s2row_{kc}")
                nc.scalar.copy(s2row[:], s2p[0:1, :])

            def emit_ln_part2(C):
                kc = C["kc"]
                ls = kc * LC
                yD0, yD1 = C["yD0"], C["yD1"]
                s1row, s2row = C["s1row"], C["s2row"]
                # stats directly on the [1, LC] row layout
                murow = rw.tile([1, LC], FP, tag="murow", name=f"murow_{kc}")
                nc.vector.tensor_scalar(murow[:], s1row[:], 1.0 / DM, None,
                                        OP.mult)
                msqr = rw.tile([1, LC], FP, tag="msqr", name=f"msqr_{kc}")
                nc.scalar.activation(msqr[:], murow[:], AF.Square)
                varr = rw.tile([1, LC], FP, tag="varr", name=f"varr_{kc}")
                nc.vector.scalar_tensor_tensor(
                    varr[:], s2row[:], 1.0 / DM, msqr[:],
                    OP.mult, OP.subtract)
                # 1/sqrt(v+eps) = exp(-0.5*ln(v+eps)): stays on the ln/exp
                # act table (Abs_reciprocal_sqrt would force a table swap)
                lnv = rw.tile([1, LC], FP, tag="lnv", name=f"lnv_{kc}")
                nc.scalar.activation(lnv[:], varr[:], AF.Ln,
                                     bias=epsc[0:1, :])
                irow = rw.tile([1, LC], FPR, tag="irow", name=f"irow_{kc}")
                nc.scalar.activation(irow[:], lnv[:], AF.Exp, scale=-0.5)
                mirow = rw.tile([1, LC], FPR, tag="mirow", name=f"mirow_{kc}")
                with nc.allow_low_precision(reason="fp32r rows for broadcast"):
                    nc.vector.tensor_tensor(mirow[:], murow[:],
                                            irow[:].bitcast(FP), OP.mult)

                # broadcast g*inv and g*mu*inv via k=1 fp32 matmuls
                ib0 = ps_pre.tile([128, LC], FP, tag="pre", name=f"ib0{kc}")
                mm512(ib0[:], cr("lngr0"), irow[:], True, True)
                yo0t = yp.tile([128, LC], FP, tag="yo0t", name=f"yo0t_{kc}")
                nc.vector.tensor_tensor(yo0t[:], yD0[:].bitcast(FP), ib0[:],
                                        OP.mult)
                mi0 = ps_pre.tile([128, LC], FP, tag="pre", name=f"mi0{kc}")
                mm512(mi0[:], cr("lngr0"), mirow[:], True, True)
                nc.vector.scalar_tensor_tensor(
                    yo0t[:], yo0t[:], cf("lnb0"), mi0[:], OP.add, OP.subtract)
                nc.sync.dma_start(yo0[:, ls:ls + LC], yo0t[:])

                ib1 = ps_pre.tile([64, LC], FP, tag="pre", name=f"ib1{kc}")
                mm512(ib1[:], cr("lngr1"), irow[:], True, True)
                yo1t = yp.tile([64, LC], FP, tag="yo1t", name=f"yo1t_{kc}")
                nc.vector.tensor_tensor(yo1t[:], yD1[:].bitcast(FP), ib1[:],
                                        OP.mult)
                mi1 = ps_pre.tile([64, LC], FP, tag="pre", name=f"mi1{kc}")
                mm512(mi1[:], cr("lngr1"), mirow[:], True, True)
                nc.vector.scalar_tensor_tensor(
                    yo1t[:], yo1t[:], cf("lnb1"), mi1[:], OP.add, OP.subtract)
                nc.sync.dma_start(yo1[:, ls:ls + LC], yo1t[:])

            # ---- software-pipelined chunk loop ----
            Ccur = emit_preamble(0)
            for kc in range(1, NCH):
                emit_gelu(kc)
            Ccur["yac0"] = ps_y.tile([128, LC], FP, tag="y", name="y0_0")
            Cfin = None
            for kc in range(NCH):
                for j in range(2):
                    emit_block(Ccur, j)
                Cnext = emit_preamble(kc + 1) if kc + 1 < NCH else None
                for j in range(2, 24):
                    if j == 6 and Cfin is not None:
                        emit_ln_part2(Cfin)
                        Cfin = None
                    emit_block(Ccur, j)
                if Cnext is not None:
                    Cnext["yac0"] = ps_y.tile([128, LC], FP, tag="y",
                                              name=f"y0_{kc + 1}")
                emit_ln_part1(Ccur)
                Cfin = Ccur
                Ccur = Cnext
            emit_ln_part2(Cfin)

    nc.compile()
    return nc


_PROG = None


def _get_prog():
    global _PROG
    if _PROG is None:
        _PROG = build_program()
    return _PROG


def _make_in_maps(inputs):
    f32 = lambda a: np.ascontiguousarray(np.asarray(a, dtype=np.float32))
    bf16 = lambda a: np.ascontiguousarray(
        np.asarray(np.asarray(a, dtype=np.float32), dtype=ml_dtypes.bfloat16))
    x = {0: f32(inputs["x_rgb"]), 1: f32(inputs["x_e"])}
    u = {0: f32(inputs["u_rgb"]), 1: f32(inputs["u_e"])}
    rw1 = {0: f32(inputs["route_rgb_w1"]), 1: f32(inputs["route_e_w1"])}
    rb1 = {0: f32(inputs["route_rgb_b1"]), 1: f32(inputs["route_e_b1"])}
    rw2 = {0: f32(inputs["route_rgb_w2"]), 1: f32(inputs["route_e_w2"])}
    rb2 = {0: f32(inputs["route_rgb_b2"]), 1: f32(inputs["route_e_b2"])}
    emb = {0: f32(inputs["emb_rgb"]), 1: f32(inputs["emb_e"])}
    tok = {0: f32(inputs["token_rgb_w"]), 1: f32(inputs["token_e_w"])}
    xproj = {0: f32(inputs["xproj_rgb"]), 1: f32(inputs["xproj_e"])}
    dtw = {0: f32(inputs["dtw_rgb"]), 1: f32(inputs["dtw_e"])}
    dtb = {0: f32(inputs["dtb_rgb"]), 1: f32(inputs["dtb_e"])}
    Alog = {0: f32(inputs["Alog_rgb"]), 1: f32(inputs["Alog_e"])}
    Dsk = {0: f32(inputs["D_rgb"]), 1: f32(inputs["D_e"])}
    lng = {0: f32(inputs["ln1_g"]), 1: f32(inputs["ln2_g"])}
    lnb = {0: f32(inputs["ln1_b"]), 1: f32(inputs["ln2_b"])}

    nmap = np.arange(P) % 16   # p -> n
    dmap = np.arange(P) // 16  # p -> d8

    in_maps = []
    for c in range(8):
        b, s = divmod(c, 2)
        o = 1 - s
        xsT = x[s][b].T.copy()          # [192, L]
        xoT = x[o][b].T.copy()
        A = -np.exp(Alog[s])            # [DM, N]
        assert np.allclose(A, A[0:1, :], atol=0), "A must be d-independent"
        Arow = A[0]                     # [N]
        WdA0 = np.zeros((16, 128, P), np.float32)
        for j in range(16):
            WdA0[j, 8 * j + dmap, np.arange(P)] = Arow[nmap]
        WdA1 = np.zeros((8, 64, P), np.float32)
        for j in range(8):
            WdA1[j, 8 * j + dmap, np.arange(P)] = Arow[nmap]
        S80 = np.zeros((16, P, 128), np.float32)
        for j in range(16):
            S80[j, np.arange(P), 8 * j + dmap] = 1.0
        S81 = np.zeros((8, P, 64), np.float32)
        for j in range(8):
            S81[j, np.arange(P), 8 * j + dmap] = 1.0
        Pm = emb[o] @ tok[o]            # [T, N]
        PmRep = np.ascontiguousarray(Pm[:, nmap])                 # [T, P]
        CrepT = np.ascontiguousarray(xproj[o][R + N:R + 2 * N][nmap].T)
        BrepT = np.ascontiguousarray((xproj[s][R:R + N][nmap]
                                      * (1.0 / Arow[nmap])[:, None]).T)
        Mdt = (dtw[s] @ xproj[s][:R]).T.copy()                    # [DM, DM]

        cb_consts = {
            "w1T0": rw1[o].T[:128], "w1T1": rw1[o].T[128:], "w2T": rw2[o].T,
            "PmRep": PmRep, "xpC0": CrepT[:128], "xpC1": CrepT[128:],
            "xpB0": BrepT[:128], "xpB1": BrepT[128:],
            "Mdt0": Mdt[:128], "Mdt1": Mdt[128:],
            "S80": np.transpose(S80, (1, 0, 2)).reshape(P, 16 * 128),
            "S81": np.transpose(S81, (1, 0, 2)).reshape(P, 8 * 64),
            "WdA0": np.transpose(WdA0, (1, 0, 2)).reshape(128, 16 * P),
            "WdA1": np.transpose(WdA1, (1, 0, 2)).reshape(64, 8 * P),
        }
        cpb_arr = np.zeros((128, CBTOT), np.float32)
        for nm, (off, r, ccols) in CBMAP.items():
            a = np.asarray(cb_consts[nm], np.float32)
            assert a.shape == (r, ccols), (nm, a.shape)
            cpb_arr[:r, off:off + ccols] = a

        cr_consts = {
            "onc0": np.ones((128, 1), np.float32),
            "onc1": np.ones((64, 1), np.float32),
            "lngr0": lng[s][None, :128], "lngr1": lng[s][None, 128:],
        }
        cpr_arr = np.zeros((128, CRTOT), np.float32)
        for nm, (off, r, ccols) in CRMAP.items():
            a = np.asarray(cr_consts[nm], np.float32)
            assert a.shape == (r, ccols), (nm, a.shape)
            cpr_arr[:r, off:off + ccols] = a

        cf_consts = {
            "identF": np.eye(128, dtype=np.float32),
            "b1c": rb1[o][:, None], "dtb0": dtb[s][:128, None],
            "dtb1": dtb[s][128:, None],
            "invA": (1.0 / Arow[nmap])[:, None],
            "Dc0": Dsk[s][:128, None], "Dc1": Dsk[s][128:, None],
            "lnb0": lnb[s][:128, None], "lnb1": lnb[s][128:, None],
        }
        cpf_arr = np.zeros((128, CFTOT), np.float32)
        for nm, (off, r, ccols) in CFMAP.items():
            a = np.asarray(cf_consts[nm], np.float32)
            assert a.shape == (r, ccols), (nm, a.shape)
            cpf_arr[:r, off:off + ccols] = a

        gqa = (-np.log(-np.log(u[o][b])) + rb2[o][None, :]).astype(np.float32)
        m = {
            "xsT0": bf16(xsT[:128]), "xsT1": bf16(xsT[128:]),
            "xoT0": bf16(xoT[:128]), "xoT1": bf16(xoT[128:]),
            "gq": gqa.reshape(L // 128, 128, T).copy(),
            "cpb": np.ascontiguousarray(cpb_arr.astype(ml_dtypes.bfloat16)),
            "cpr": cpr_arr,
            "cpf": cpf_arr,
        }
        in_maps.append(m)
    return in_maps


def run(inputs, trace=False):
    nc = _get_prog()
    in_maps = _make_in_maps(inputs)
    res = run_bass_kernel_spmd(nc, in_maps, list(range(8)), trace=trace)
    out = np.zeros((2, B, DM, L), np.float32)
    for c in range(8):
        b, s = divmod(c, 2)
        out[s, b, :128] = res.results[c]["yo0"]
        out[s, b, 128:] = res.results[c]["yo1"]
    return out, res


def kernel(**inputs):
    out, _ = run(inputs, trace=False)
    return out



# revision 29
# speedup vs baseline: 1.0090x; 1.0090x over previous
"""Cross-modal selective-scan (ASSM) kernel for 8 TRN2 NeuronCores.

Sharding: one core per (batch, stream) pair: core = b*2 + s, s=0 rgb / s=1 e.
Each core computes the full forward for its stream (routing+gumbel of the
OTHER stream feeds C — cross-modal), the L=4096 selective scan over
(D=192, N=16) states, and the output layernorm. Outputs are gathered on host.

v3 highlights:
 - matmuls in bf16 (x-projections / routing / yacc) and fp32r (scan args),
   never plain fp32 on big streams -> ~4x PE throughput per column.
 - the state-injection/readout chain (dBu, h, ym, B, C) runs in bf16: DVE
   2x mode for the elementwise ops, half the SBUF traffic. The compounding
   decay dA stays fp32 (bf16 decay quantization accumulates over the scan).
 - gumbel noise -ln(-ln u) (+ b2 routing bias) precomputed on host.
 - the next chunk's load/routing/projection preamble is emitted in the
   middle of the current chunk's scan phase (software pipelining).
 - GpSimd handles SBUF-only side ops (w-build, softplus add, one-hot eq,
   y^2); it cannot access PSUM, so PSUM readers stay on Vector/Scalar.
"""

import numpy as np
import ml_dtypes

import concourse.bass as bass
import concourse.bacc as bacc
import concourse.mybir as mybir
import concourse.tile as tile
from concourse.bass_utils import run_bass_kernel_spmd

FP = mybir.dt.float32
FPR = mybir.dt.float32r
BF = mybir.dt.bfloat16
OP = mybir.AluOpType
AF = mybir.ActivationFunctionType
F16 = mybir.dt.float16

B, L, DM, N, R, T, H3 = 4, 4096, 192, 16, 12, 64, 64
P = 128
LC = 1024               # chunk along L
NCH = L // LC           # 4
SUB = 512
LEPS = 1e-5
NT = LC // 128          # 8 token tiles per chunk
# GpSimd offload measured SLOWER: its TT is ~2.7us/[128,1024] and it
# contends with DVE for SBUF ports (scans slowed 2326->2968ns). Keep all
# dBu/ym mults on DVE; a few dBu read PSUM directly (saves Act evac time).
DBU_DVE = {2, 8, 14, 20}

# ---- packed-constant layouts: name -> (col offset, rows, cols) ----
def _pack(shapes):
    m, off = {}, 0
    for nm, r, c in shapes:
        m[nm] = (off, r, c)
        off += c
    return m, off

_CB_SHAPES = [
    ("w1T0", 128, 64), ("w1T1", 64, 64), ("w2T", 64, 64), ("PmRep", 64, 128),
    ("xpC0", 128, 128), ("xpC1", 64, 128), ("xpB0", 128, 128),
    ("xpB1", 64, 128), ("Mdt0", 128, 192), ("Mdt1", 64, 192),
    ("S80", 128, 16 * 128), ("S81", 128, 8 * 64),
    ("WpA", 128, 24 * 128),
]
CBMAP, CBTOT = _pack(_CB_SHAPES)

_CR_SHAPES = [
    ("onc0", 128, 1), ("onc1", 64, 1), ("lngr0", 1, 128), ("lngr1", 1, 64),
]
CRMAP, CRTOT = _pack(_CR_SHAPES)

_CF_SHAPES = [
    ("identF", 128, 128), ("b1c", 64, 1), ("dtb0", 128, 1), ("dtb1", 64, 1),
    ("invA", 128, 1), ("Dc0", 128, 1), ("Dc1", 64, 1), ("lnb0", 128, 1),
    ("lnb1", 64, 1),
]
CFMAP, CFTOT = _pack(_CF_SHAPES)


def build_program():
    nc = bacc.Bacc("TRN2", target_bir_lowering=False, debug=False)

    xsT0 = nc.declare_dram_parameter("xsT0", [128, L], BF, isOutput=False)
    xsT1 = nc.declare_dram_parameter("xsT1", [64, L], BF, isOutput=False)
    xoT0 = nc.declare_dram_parameter("xoT0", [128, L], BF, isOutput=False)
    xoT1 = nc.declare_dram_parameter("xoT1", [64, L], BF, isOutput=False)
    gq = nc.declare_dram_parameter("gq", [L // 128, 128, T], FP, isOutput=False)
    cpb = nc.declare_dram_parameter("cpb", [128, CBTOT], BF, isOutput=False)
    cpr = nc.declare_dram_parameter("cpr", [128, CRTOT], FPR, isOutput=False)
    cpf = nc.declare_dram_parameter("cpf", [128, CFTOT], FP, isOutput=False)
    yo0 = nc.declare_dram_parameter("yo0", [128, L], FP, isOutput=True)
    yo1 = nc.declare_dram_parameter("yo1", [64, L], FP, isOutput=True)

    with tile.TileContext(nc) as tc:
        with (
            tc.tile_pool(name="const", bufs=1) as cp,
            tc.tile_pool(name="xin", bufs=2) as xp,
            tc.tile_pool(name="dwp", bufs=2) as dwp,
            tc.tile_pool(name="proj", bufs=2) as pj,
            tc.tile_pool(name="route", bufs=2) as rt,
            tc.tile_pool(name="spool", bufs=2) as sp_,
            tc.tile_pool(name="blk", bufs=3) as bk,
            tc.tile_pool(name="hpool", bufs=3) as hp,
            tc.tile_pool(name="ypool", bufs=1) as yp,
            tc.tile_pool(name="rows", bufs=1) as rw,
            tc.tile_pool(name="persist", bufs=1) as pe_,
            tc.tile_pool(name="ps_scan", bufs=2, space="PSUM") as ps_scan,
            tc.tile_pool(name="ps_pre", bufs=1, space="PSUM") as ps_pre,
            tc.tile_pool(name="ps_y", bufs=1, space="PSUM") as ps_y,
        ):
            cbk = cp.tile([128, CBTOT], BF, tag="cpb")
            nc.sync.dma_start(cbk[:], cpb[:])
            crk = cp.tile([128, CRTOT], FPR, tag="cpr")
            nc.sync.dma_start(crk[:], cpr[:])
            cfk = cp.tile([128, CFTOT], FP, tag="cpf")
            nc.sync.dma_start(cfk[:], cpf[:])

            def cb(name):
                off, r, c = CBMAP[name]
                return cbk[0:r, off:off + c]

            def cr(name):
                off, r, c = CRMAP[name]
                return crk[0:r, off:off + c]

            def cf(name):
                off, r, c = CFMAP[name]
                return cfk[0:r, off:off + c]

            def mm512(out, lhsT, rhs, start, stop):
                # ISA caps the moving operand at 512 elements per matmul
                n = rhs.shape[-1]
                for q in range(0, n, 512):
                    e = min(q + 512, n)
                    nc.tensor.matmul(out[:, q:e], lhsT, rhs[:, q:e],
                                     start=start, stop=stop)

            hlast = pe_.tile([P, 24], FP)
            epsc = pe_.tile([128, 1], FP)
            nc.vector.memset(epsc[:], LEPS)

            # xo stays resident for the whole kernel (routing + C-projection)
            xo0a = pe_.tile([128, L], BF, tag="xo0a")
            nc.sync.dma_start(xo0a[:], xoT0[:])
            xo1a = pe_.tile([64, L], BF, tag="xo1a")
            nc.sync.dma_start(xo1a[:], xoT1[:])
            hgk = {}

            def emit_gelu(kc):
                # routing hidden layer for chunk kc (gelu act table)
                ls = kc * LC
                zt = ps_pre.tile([128, LC], FP, tag="pre", name=f"zt{kc}")
                mm512(zt[0:H3, :], cb("w1T0"), xo0a[:, ls:ls + LC], True, False)
                mm512(zt[0:H3, :], cb("w1T1"), xo1a[:, ls:ls + LC], False, True)
                hg = rt.tile([H3, LC], BF, tag="hg", bufs=4, name=f"hg_{kc}")
                nc.scalar.activation(hg[:], zt[0:H3, :], AF.Gelu,
                                     bias=cf("b1c"))
                hgk[kc] = hg

            def emit_preamble(kc):
                """Loads + projections + routing + w-build for chunk kc."""
                ls = kc * LC
                c0 = ls // 128
                C = {}
                xs0 = C["xs0"] = xp.tile([128, LC], BF, tag="xs0", name=f"xs0_{kc}")
                xs1 = C["xs1"] = xp.tile([64, LC], BF, tag="xs1", name=f"xs1_{kc}")
                gt = xp.tile([128, NT * T], FP, tag="gt", name=f"gt_{kc}")
                nc.sync.dma_start(xs0[:], xsT0[:, ls:ls + LC])
                nc.sync.dma_start(xs1[:], xsT1[:, ls:ls + LC])
                nc.sync.dma_start(
                    gt[:].rearrange("p (c t) -> p c t", c=NT),
                    gq[c0:c0 + NT].rearrange("c p t -> p c t"))

                # dt -> softplus -> dl ; w = dl*x   (dw = [dl | w])
                dw0 = C["dw0"] = dwp.tile([128, 2 * LC], BF, tag="dw0", name=f"dw0_{kc}")
                dw1 = C["dw1"] = dwp.tile([64, 2 * LC], BF, tag="dw1", name=f"dw1_{kc}")
                dtp0 = ps_pre.tile([128, LC], FP, tag="pre", name=f"dt0{kc}")
                mm512(dtp0[:], cb("Mdt0")[:, 0:128], xs0[:], True, False)
                mm512(dtp0[:], cb("Mdt1")[:, 0:128], xs1[:], False, True)
                # softplus(x) = ln(exp(x) + 1); x = dt + dtb stays < ~3 here.
                # Both Exp ops before both Ln ops: the act tables the
                # compiler picks keep exp and ln apart, so interleaving
                # would cost a 1.3us table swap per switch.
                sp0 = sp_.tile([128, LC], FP, tag="sp0", name=f"sp0_{kc}")
                nc.scalar.activation(sp0[:], dtp0[:], AF.Exp, bias=cf("dtb0"))
                dtp1 = ps_pre.tile([64, LC], FP, tag="pre", name=f"dt1{kc}")
                mm512(dtp1[:], cb("Mdt0")[:, 128:DM], xs0[:], True, False)
                mm512(dtp1[:], cb("Mdt1")[:, 128:DM], xs1[:], False, True)
                sp1 = sp_.tile([64, LC], FP, tag="sp1", name=f"sp1_{kc}")
                nc.scalar.activation(sp1[:], dtp1[:], AF.Exp, bias=cf("dtb1"))
                nc.scalar.activation(dw0[:, 0:LC], sp0[:], AF.Ln, bias=1.0)
                nc.scalar.activation(dw1[:, 0:LC], sp1[:], AF.Ln, bias=1.0)
                nc.gpsimd.tensor_tensor(dw0[:, LC:2 * LC], dw0[:, 0:LC],
                                        xs0[:], OP.mult)
                nc.gpsimd.tensor_tensor(dw1[:, LC:2 * LC], dw1[:, 0:LC],
                                        xs1[:], OP.mult)

                # Brep and Crep live side by side in one [128, 2LC] tile so
                # the per-block dBu/ym multiplies can fuse into a single
                # [128, 2LC] DVE op (halves the per-op overhead)
                bc = C["bc"] = pj.tile([128, 2 * LC], BF, tag="bc", name=f"bc_{kc}")
                bp = ps_pre.tile([128, LC], FP, tag="pre", name=f"bp{kc}")
                mm512(bp[:], cb("xpB0"), xs0[:], True, False)
                mm512(bp[:], cb("xpB1"), xs1[:], False, True)
                nc.scalar.copy(bc[:, 0:LC], bp[:])

                # routing of the other stream -> one-hot OT
                if kc not in hgk:
                    emit_gelu(kc)
                hg = hgk[kc]
                z2 = ps_scan.tile([128, NT * T], FP, tag="scan", name=f"z2{kc}")
                for i in range(NT):
                    nc.tensor.matmul(z2[:, i * T:(i + 1) * T],
                                     hg[:, i * 128:(i + 1) * 128], cb("w2T"),
                                     start=True, stop=True)
                zg = rt.tile([128, NT * T], FP, tag="zg", name=f"zg_{kc}")
                nc.vector.tensor_tensor(zg[:], z2[:], gt[:], OP.add)
                oh = rt.tile([128, NT * T], FP, tag="oh", name=f"oh_{kc}")
                for i in range(NT):
                    sl = slice(i * T, (i + 1) * T)
                    m8 = rt.tile([128, 8], FP, tag="m8", bufs=3, name=f"m8_{kc}_{i}")
                    nc.vector.max(m8[:], zg[:, sl])
                    # is_equal on GpSimd (SBUF-only op; keeps DVE free for scans)
                    nc.gpsimd.tensor_scalar(oh[:, sl], zg[:, sl], m8[:, 0:1],
                                            None, OP.is_equal)
                tp = ps_pre.tile([128, LC], FP, tag="pre", name=f"tp{kc}")
                for i in range(NT):
                    nc.tensor.transpose(tp[0:T, i * 128:(i + 1) * 128],
                                        oh[:, i * T:(i + 1) * T], cf("identF"))
                OT = rt.tile([T, LC], BF, tag="OT", name=f"OT_{kc}")
                nc.scalar.copy(OT[:], tp[0:T, :])

                cpp = ps_pre.tile([128, LC], FP, tag="pre", name=f"cp{kc}")
                mm512(cpp[:], cb("xpC0"), xo0a[:, ls:ls + LC], True, False)
                mm512(cpp[:], cb("xpC1"), xo1a[:, ls:ls + LC], False, False)
                mm512(cpp[:], cb("PmRep"), OT[:], False, True)
                nc.scalar.copy(bc[:, LC:2 * LC], cpp[:])
                C["kc"] = kc
                C["hl_pend"] = []
                C["yacc_pend"] = []
                C["PT"] = {}
                C["pend_ym"] = {}
                C["dAp"] = {}
                C["mmBp"] = {}
                return C


            def emit_yacc(C):
                kc = C["kc"]
                j, sc, rows_, first, last = C["yacc_pend"].pop(0)
                ym = C["pend_ym"].pop(j)
                yac = C["yac0"] if j < 16 else C["yac1"]
                mm512(yac[0:rows_, :], sc, ym, first, last)
                if j == 15:
                    yD0 = C["yD0"] = yp.tile([128, LC], FPR, tag="yD0",
                                             name=f"yD0_{kc}")
                    nc.vector.scalar_tensor_tensor(
                        yD0[:], C["xs0"][:], cf("Dc0"), C["yac0"][:],
                        OP.mult, OP.add)
                    C["yac1"] = ps_y.tile([64, LC], FP, tag="y",
                                          name=f"y1_{kc}")

            def _blk_geo(j):
                """(dw tile key, 32-aligned window base == PE row position)
                for block j: the stationary and moving operands must both
                start at the row-group partition, so concurrency pairs come
                from different 32-channel groups."""
                if j < 16:
                    return "dw0", 32 * (j // 4)
                return "dw1", 32 * ((j - 16) // 4)

            def emit_pair_mm(C, ja, jb):
                """mmpA/mmpB for blocks ja, jb as CONCURRENT k=32 matmuls on
                distinct PE row-groups (tile_position): each block's
                broadcast only contracts over 8 real channels, and ja/jb sit
                in different 32-partition groups, so the two matmuls overlap
                on the PE's independent 32-row sub-array strips (~2x)."""
                kc = C["kc"]
                mmA, mmB = {}, {}
                for j in (ja, jb):
                    mmA[j] = ps_scan.tile([128, LC], FP, tag="scan",
                                          name=f"mmA{kc}_{j}")
                for p0 in range(0, LC, 512):
                    for j in (ja, jb):
                        dwt, wb = _blk_geo(j)
                        wa = cb("WpA")[wb:wb + 32,
                                       j * 128:(j + 1) * 128]
                        nc.tensor.matmul(
                            mmA[j][:, p0:p0 + 512], wa,
                            C[dwt][wb:wb + 32, p0:p0 + 512],
                            start=True, stop=True, tile_position=(wb, 0))
                for j in (ja, jb):
                    dA = bk.tile([P, LC], F16, tag="dA", bufs=4,
                                 name=f"dA_{kc}_{j}")
                    nc.scalar.activation(dA[:], mmA[j][:], AF.Exp)
                    C["dAp"][j] = dA
                for j in (ja, jb):
                    mmB[j] = ps_scan.tile([128, LC], FP, tag="scan",
                                          name=f"mmB{kc}_{j}")
                for p0 in range(0, LC, 512):
                    for j in (ja, jb):
                        dwt, wb = _blk_geo(j)
                        wa = cb("WpA")[wb:wb + 32,
                                       j * 128:(j + 1) * 128]
                        nc.tensor.matmul(
                            mmB[j][:, p0:p0 + 512], wa,
                            C[dwt][wb:wb + 32, LC + p0:LC + p0 + 512],
                            start=True, stop=True, tile_position=(wb, 0))
                C["mmBp"][ja], C["mmBp"][jb] = mmB[ja], mmB[jb]

            def emit_block(C, j, jprev):
                kc = C["kc"]
                if j < 16:
                    sc = cb("S80")[:, j * 128:(j + 1) * 128]
                    rows_ = P
                    first, last = j == 0, j == 15
                else:
                    sc = cb("S81")[:, (j - 16) * 64:(j - 15) * 64]
                    rows_ = 64
                    first, last = j == 16, j == 23
                # deferred hlast copies (Act) — 2 blocks late so Act never
                # stalls waiting for the scan of the current block
                while len(C["hl_pend"]) > 2:
                    _, jj, hh = C["hl_pend"].pop(0)
                    nc.scalar.copy(hlast[:, jj:jj + 1], hh)
                dA = C["dAp"].pop(j)
                mmpB = C["mmBp"].pop(j)
                bc = C["bc"]
                # paired elementwise: wcp(j) lands next to h(jprev) in the
                # previously-executed block's PT tile, so ONE [128, 2LC]
                # multiply against [Brep|Crep] yields dBu(j) + ym(jprev)
                if jprev is None:
                    wcp = bk.tile([P, LC], BF, tag="wcp", bufs=2, name=f"wcp_{kc}_0")
                    nc.scalar.copy(wcp[:], mmpB[:])
                    dBu0 = bk.tile([P, LC], BF, tag="dBu", bufs=2, name=f"dBu_{kc}_0")
                    nc.vector.tensor_tensor(dBu0[:], wcp[:], bc[:, 0:LC],
                                            OP.mult)
                    data1 = dBu0[:]
                else:
                    PTp = C["PT"].pop(jprev)
                    nc.scalar.copy(PTp[:, 0:LC], mmpB[:])
                    out2 = hp.tile([P, 2 * LC], BF, tag="out2", bufs=3,
                                   name=f"out2_{kc}_{j}")
                    nc.vector.tensor_tensor(out2[:], PTp[:], bc[:], OP.mult)
                    C["pend_ym"][jprev] = out2[:, LC:2 * LC]
                    data1 = out2[:, 0:LC]
                PT = hp.tile([P, 2 * LC], BF, tag="PT", bufs=3,
                             name=f"PT_{kc}_{j}")
                C["PT"][j] = PT
                init = 0.0 if kc == 0 else hlast[:, j:j + 1]
                nc.vector.tensor_tensor_scan(PT[:, LC:2 * LC], dA[:], data1,
                                             init, OP.mult, OP.add)
                if kc < NCH - 1:
                    C["hl_pend"].append((j, j, PT[:, 2 * LC - 1:2 * LC]))
                if j == 23:
                    ymL = hp.tile([P, LC], BF, tag="ymL", name=f"ymL_{kc}")
                    nc.vector.tensor_tensor(ymL[:], PT[:, LC:2 * LC],
                                            bc[:, LC:2 * LC], OP.mult)
                    C["pend_ym"][23] = ymL[:]
                # yacc matmuls run late so PE never waits on the DVE scan
                # pipeline mid-stream (HAM throttle avoidance); ym(j) only
                # exists after block j+1's paired multiply
                C["yacc_pend"].append((j, sc, rows_, first, last))
                while len(C["yacc_pend"]) > 2:
                    emit_yacc(C)

            def emit_ln_part1(C):
                """Yacc drain + row sums for chunk C. The DVE-dependent tail
                (emit_ln_part2) is deferred into the next chunk's block loop
                so DVE's FIFO isn't blocked on the PE sum matmuls while the
                next chunk's scans are ready to run."""
                kc = C["kc"]
                while C["yacc_pend"]:
                    emit_yacc(C)
                while C["hl_pend"]:
                    _, jj, hh = C["hl_pend"].pop(0)
                    nc.scalar.copy(hlast[:, jj:jj + 1], hh)
                yD0 = C["yD0"]
                yD1 = C["yD1"] = yp.tile([64, LC], FPR, tag="yD1",
                                         name=f"yD1_{kc}")
                nc.vector.scalar_tensor_tensor(
                    yD1[:], C["xs1"][:], cf("Dc1"), C["yac1"][:],
                    OP.mult, OP.add)
                ysq0 = yp.tile([128, LC], FPR, tag="ysq0", name=f"ysq0_{kc}")
                nc.scalar.activation(ysq0[:], yD0[:].bitcast(FP), AF.Square)
                ysq1 = yp.tile([64, LC], FPR, tag="ysq1", name=f"ysq1_{kc}")
                nc.scalar.activation(ysq1[:], yD1[:].bitcast(FP), AF.Square)

                # sums live in the "pre" ring: the preamble is done with it
                # here, and polluting the "scan" ring would stall the next
                # chunk's mmpA/mmpB allocations behind the LN reads
                s1p = ps_pre.tile([128, LC], FP, tag="pre", name=f"s1{kc}")
                mm512(s1p[0:1, :], cr("onc0"), yD0[:], True, False)
                mm512(s1p[0:1, :], cr("onc1"), yD1[:], False, True)
                s1row = C["s1row"] = rw.tile([1, LC], FP, tag="s1row",
                                             name=f"s1row_{kc}")
                nc.scalar.copy(s1row[:], s1p[0:1, :])
                s2p = ps_pre.tile([128, LC], FP, tag="pre", name=f"s2{kc}")
                mm512(s2p[0:1, :], cr("onc0"), ysq0[:], True, False)
                mm512(s2p[0:1, :], cr("onc1"), ysq1[:], False, True)
                s2row = C["s2row"] = rw.tile([1, LC], FP, tag="s2row",
                                             name=f"s2row_{kc}")
                nc.scalar.copy(s2row[:], s2p[0:1, :])

            def emit_ln_part2(C):
                kc = C["kc"]
                ls = kc * LC
                yD0, yD1 = C["yD0"], C["yD1"]
                s1row, s2row = C["s1row"], C["s2row"]
                # stats directly on the [1, LC] row layout
                murow = rw.tile([1, LC], FP, tag="murow", name=f"murow_{kc}")
                nc.vector.tensor_scalar(murow[:], s1row[:], 1.0 / DM, None,
                                        OP.mult)
                msqr = rw.tile([1, LC], FP, tag="msqr", name=f"msqr_{kc}")
                nc.scalar.activation(msqr[:], murow[:], AF.Square)
                varr = rw.tile([1, LC], FP, tag="varr", name=f"varr_{kc}")
                nc.vector.scalar_tensor_tensor(
                    varr[:], s2row[:], 1.0 / DM, msqr[:],
                    OP.mult, OP.subtract)
                # 1/sqrt(v+eps) = exp(-0.5*ln(v+eps)): stays on the ln/exp
                # act table (Abs_reciprocal_sqrt would force a table swap)
                lnv = rw.tile([1, LC], FP, tag="lnv", name=f"lnv_{kc}")
                nc.scalar.activation(lnv[:], varr[:], AF.Ln,
                                     bias=epsc[0:1, :])
                irow = rw.tile([1, LC], FPR, tag="irow", name=f"irow_{kc}")
                nc.scalar.activation(irow[:], lnv[:], AF.Exp, scale=-0.5)
                mirow = rw.tile([1, LC], FPR, tag="mirow", name=f"mirow_{kc}")
                with nc.allow_low_precision(reason="fp32r rows for broadcast"):
                    nc.vector.tensor_tensor(mirow[:], murow[:],
                                            irow[:].bitcast(FP), OP.mult)

                # broadcast g*inv and g*mu*inv via k=1 fp32 matmuls
                ib0 = ps_pre.tile([128, LC], FP, tag="pre", name=f"ib0{kc}")
                mm512(ib0[:], cr("lngr0"), irow[:], True, True)
                yo0t = yp.tile([128, LC], FP, tag="yo0t", name=f"yo0t_{kc}")
                nc.vector.tensor_tensor(yo0t[:], yD0[:].bitcast(FP), ib0[:],
                                        OP.mult)
                mi0 = ps_pre.tile([128, LC], FP, tag="pre", name=f"mi0{kc}")
                mm512(mi0[:], cr("lngr0"), mirow[:], True, True)
                nc.vector.scalar_tensor_tensor(
                    yo0t[:], yo0t[:], cf("lnb0"), mi0[:], OP.add, OP.subtract)
                nc.sync.dma_start(yo0[:, ls:ls + LC], yo0t[:])

                ib1 = ps_pre.tile([64, LC], FP, tag="pre", name=f"ib1{kc}")
                mm512(ib1[:], cr("lngr1"), irow[:], True, True)
                yo1t = yp.tile([64, LC], FP, tag="yo1t", name=f"yo1t_{kc}")
                nc.vector.tensor_tensor(yo1t[:], yD1[:].bitcast(FP), ib1[:],
                                        OP.mult)
                mi1 = ps_pre.tile([64, LC], FP, tag="pre", name=f"mi1{kc}")
                mm512(mi1[:], cr("lngr1"), mirow[:], True, True)
                nc.vector.scalar_tensor_tensor(
                    yo1t[:], yo1t[:], cf("lnb1"), mi1[:], OP.add, OP.subtract)
                nc.sync.dma_start(yo1[:, ls:ls + LC], yo1t[:])

            # block execution order: concurrency pairs (j, j+4) come from
            # different 32-channel groups so their small-k matmuls overlap
            # on distinct PE row-group strips
            SEQ = [0, 4, 1, 5, 2, 6, 3, 7, 8, 12, 9, 13, 10, 14, 11, 15,
                   16, 20, 17, 21, 18, 22, 19, 23]

            # ---- software-pipelined chunk loop ----
            Ccur = emit_preamble(0)
            for kc in range(1, NCH):
                emit_gelu(kc)
            Ccur["yac0"] = ps_y.tile([128, LC], FP, tag="y", name="y0_0")
            Cfin = None
            for kc in range(NCH):
                emit_pair_mm(Ccur, SEQ[0], SEQ[1])
                emit_block(Ccur, SEQ[0], None)
                emit_block(Ccur, SEQ[1], SEQ[0])
                Cnext = emit_preamble(kc + 1) if kc + 1 < NCH else None
                for i in range(2, 24, 2):
                    if i == 6 and Cfin is not None:
                        emit_ln_part2(Cfin)
                        Cfin = None
                    emit_pair_mm(Ccur, SEQ[i], SEQ[i + 1])
                    emit_block(Ccur, SEQ[i], SEQ[i - 1])
                    emit_block(Ccur, SEQ[i + 1], SEQ[i])
                if Cnext is not None:
                    Cnext["yac0"] = ps_y.tile([128, LC], FP, tag="y",
                                              name=f"y0_{kc + 1}")
                emit_ln_part1(Ccur)
                Cfin = Ccur
                Ccur = Cnext
            emit_ln_part2(Cfin)

    nc.compile()
    return nc


_PROG = None


def _get_prog():
    global _PROG
    if _PROG is None:
        _PROG = build_program()
    return _PROG


def _make_in_maps(inputs):
    f32 = lambda a: np.ascontiguousarray(np.asarray(a, dtype=np.float32))
    bf16 = lambda a: np.ascontiguousarray(
        np.asarray(np.asarray(a, dtype=np.float32), dtype=ml_dtypes.bfloat16))
    x = {0: f32(inputs["x_rgb"]), 1: f32(inputs["x_e"])}
    u = {0: f32(inputs["u_rgb"]), 1: f32(inputs["u_e"])}
    rw1 = {0: f32(inputs["route_rgb_w1"]), 1: f32(inputs["route_e_w1"])}
    rb1 = {0: f32(inputs["route_rgb_b1"]), 1: f32(inputs["route_e_b1"])}
    rw2 = {0: f32(inputs["route_rgb_w2"]), 1: f32(inputs["route_e_w2"])}
    rb2 = {0: f32(inputs["route_rgb_b2"]), 1: f32(inputs["route_e_b2"])}
    emb = {0: f32(inputs["emb_rgb"]), 1: f32(inputs["emb_e"])}
    tok = {0: f32(inputs["token_rgb_w"]), 1: f32(inputs["token_e_w"])}
    xproj = {0: f32(inputs["xproj_rgb"]), 1: f32(inputs["xproj_e"])}
    dtw = {0: f32(inputs["dtw_rgb"]), 1: f32(inputs["dtw_e"])}
    dtb = {0: f32(inputs["dtb_rgb"]), 1: f32(inputs["dtb_e"])}
    Alog = {0: f32(inputs["Alog_rgb"]), 1: f32(inputs["Alog_e"])}
    Dsk = {0: f32(inputs["D_rgb"]), 1: f32(inputs["D_e"])}
    lng = {0: f32(inputs["ln1_g"]), 1: f32(inputs["ln2_g"])}
    lnb = {0: f32(inputs["ln1_b"]), 1: f32(inputs["ln2_b"])}

    nmap = np.arange(P) % 16   # p -> n
    dmap = np.arange(P) // 16  # p -> d8

    in_maps = []
    for c in range(8):
        b, s = divmod(c, 2)
        o = 1 - s
        xsT = x[s][b].T.copy()          # [192, L]
        xoT = x[o][b].T.copy()
        A = -np.exp(Alog[s])            # [DM, N]
        assert np.allclose(A, A[0:1, :], atol=0), "A must be d-independent"
        Arow = A[0]                     # [N]
        # small-k stationaries at the partitions of their moving window:
        # block j's 8 channels sit at dw-tile rows 8*jl+dmap (jl = local
        # block index within its dw tile), which is also the PE row-group
        # position fed to tile_position
        WpA = np.zeros((24, 128, P), np.float32)
        for j in range(24):
            jl = j if j < 16 else j - 16
            WpA[j, 8 * jl + dmap, np.arange(P)] = Arow[nmap]
        S80 = np.zeros((16, P, 128), np.float32)
        for j in range(16):
            S80[j, np.arange(P), 8 * j + dmap] = 1.0
        S81 = np.zeros((8, P, 64), np.float32)
        for j in range(8):
            S81[j, np.arange(P), 8 * j + dmap] = 1.0
        Pm = emb[o] @ tok[o]            # [T, N]
        PmRep = np.ascontiguousarray(Pm[:, nmap])                 # [T, P]
        CrepT = np.ascontiguousarray(xproj[o][R + N:R + 2 * N][nmap].T)
        BrepT = np.ascontiguousarray((xproj[s][R:R + N][nmap]
                                      * (1.0 / Arow[nmap])[:, None]).T)
        Mdt = (dtw[s] @ xproj[s][:R]).T.copy()                    # [DM, DM]

        cb_consts = {
            "w1T0": rw1[o].T[:128], "w1T1": rw1[o].T[128:], "w2T": rw2[o].T,
            "PmRep": PmRep, "xpC0": CrepT[:128], "xpC1": CrepT[128:],
            "xpB0": BrepT[:128], "xpB1": BrepT[128:],
            "Mdt0": Mdt[:128], "Mdt1": Mdt[128:],
            "S80": np.transpose(S80, (1, 0, 2)).reshape(P, 16 * 128),
            "S81": np.transpose(S81, (1, 0, 2)).reshape(P, 8 * 64),
            "WpA": np.transpose(WpA, (1, 0, 2)).reshape(128, 24 * P),
        }
        cpb_arr = np.zeros((128, CBTOT), np.float32)
        for nm, (off, r, ccols) in CBMAP.items():
            a = np.asarray(cb_consts[nm], np.float32)
            assert a.shape == (r, ccols), (nm, a.shape)
            cpb_arr[:r, off:off + ccols] = a

        cr_consts = {
            "onc0": np.ones((128, 1), np.float32),
            "onc1": np.ones((64, 1), np.float32),
            "lngr0": lng[s][None, :128], "lngr1": lng[s][None, 128:],
        }
        cpr_arr = np.zeros((128, CRTOT), np.float32)
        for nm, (off, r, ccols) in CRMAP.items():
            a = np.asarray(cr_consts[nm], np.float32)
            assert a.shape == (r, ccols), (nm, a.shape)
            cpr_arr[:r, off:off + ccols] = a

        cf_consts = {
            "identF": np.eye(128, dtype=np.float32),
            "b1c": rb1[o][:, None], "dtb0": dtb[s][:128, None],
            "dtb1": dtb[s][128:, None],
            "invA": (1.0 / Arow[nmap])[:, None],
            "Dc0": Dsk[s][:128, None], "Dc1": Dsk[s][128:, None],
            "lnb0": lnb[s][:128, None], "lnb1": lnb[s][128:, None],
        }
        cpf_arr = np.zeros((128, CFTOT), np.float32)
        for nm, (off, r, ccols) in CFMAP.items():
            a = np.asarray(cf_consts[nm], np.float32)
            assert a.shape == (r, ccols), (nm, a.shape)
            cpf_arr[:r, off:off + ccols] = a

        gqa = (-np.log(-np.log(u[o][b])) + rb2[o][None, :]).astype(np.float32)
        m = {
            "xsT0": bf16(xsT[:128]), "xsT1": bf16(xsT[128:]),
            "xoT0": bf16(xoT[:128]), "xoT1": bf16(xoT[128:]),
            "gq": gqa.reshape(L // 128, 128, T).copy(),
            "cpb": np.ascontiguousarray(cpb_arr.astype(ml_dtypes.bfloat16)),
            "cpr": cpr_arr,
            "cpf": cpf_arr,
        }
        in_maps.append(m)
    return in_maps


def run(inputs, trace=False):
    nc = _get_prog()
    in_maps = _make_in_maps(inputs)
    res = run_bass_kernel_spmd(nc, in_maps, list(range(8)), trace=trace)
    out = np.zeros((2, B, DM, L), np.float32)
    for c in range(8):
        b, s = divmod(c, 2)
        out[s, b, :128] = res.results[c]["yo0"]
        out[s, b, 128:] = res.results[c]["yo1"]
    return out, res


def kernel(**inputs):
    out, _ = run(inputs, trace=False)
    return out



# revision 30
# speedup vs baseline: 1.0482x; 1.0388x over previous
"""Cross-modal selective-scan (ASSM) kernel for 8 TRN2 NeuronCores.

Sharding: one core per (batch, stream) pair: core = b*2 + s, s=0 rgb / s=1 e.
Each core computes the full forward for its stream (routing+gumbel of the
OTHER stream feeds C — cross-modal), the L=4096 selective scan over
(D=192, N=16) states, and the output layernorm. Outputs are gathered on host.

v3 highlights:
 - matmuls in bf16 (x-projections / routing / yacc) and fp32r (scan args),
   never plain fp32 on big streams -> ~4x PE throughput per column.
 - the state-injection/readout chain (dBu, h, ym, B, C) runs in bf16: DVE
   2x mode for the elementwise ops, half the SBUF traffic. The compounding
   decay dA stays fp32 (bf16 decay quantization accumulates over the scan).
 - gumbel noise -ln(-ln u) (+ b2 routing bias) precomputed on host.
 - the next chunk's load/routing/projection preamble is emitted in the
   middle of the current chunk's scan phase (software pipelining).
 - GpSimd handles SBUF-only side ops (w-build, softplus add, one-hot eq,
   y^2); it cannot access PSUM, so PSUM readers stay on Vector/Scalar.
"""

import numpy as np
import ml_dtypes

import concourse.bass as bass
import concourse.bacc as bacc
import concourse.mybir as mybir
import concourse.tile as tile
from concourse.bass_utils import run_bass_kernel_spmd

FP = mybir.dt.float32
FPR = mybir.dt.float32r
BF = mybir.dt.bfloat16
OP = mybir.AluOpType
AF = mybir.ActivationFunctionType
F16 = mybir.dt.float16

B, L, DM, N, R, T, H3 = 4, 4096, 192, 16, 12, 64, 64
P = 128
LC = 1024               # chunk along L
NCH = L // LC           # 4
SUB = 512
LEPS = 1e-5
NT = LC // 128          # 8 token tiles per chunk
DBU_DVE = {2, 8, 14, 20}   # blocks whose dBu reads PSUM directly on Vector

# ---- packed-constant layouts: name -> (col offset, rows, cols) ----
def _pack(shapes):
    m, off = {}, 0
    for nm, r, c in shapes:
        m[nm] = (off, r, c)
        off += c
    return m, off

_CB_SHAPES = [
    ("w1T0", 128, 64), ("w1T1", 64, 64), ("w2T", 64, 64), ("PmRep", 64, 128),
    ("xpC0", 128, 128), ("xpC1", 64, 128), ("xpB0", 128, 128),
    ("xpB1", 64, 128), ("Mdt0", 128, 192), ("Mdt1", 64, 192),
    ("S80", 128, 16 * 128), ("S81", 128, 8 * 64),
    ("WdA0", 128, 16 * 128), ("WdA1", 64, 8 * 128),
]
CBMAP, CBTOT = _pack(_CB_SHAPES)

_CR_SHAPES = [
    ("onc0", 128, 1), ("onc1", 64, 1), ("lngr0", 1, 128), ("lngr1", 1, 64),
]
CRMAP, CRTOT = _pack(_CR_SHAPES)

_CF_SHAPES = [
    ("identF", 128, 128), ("b1c", 64, 1), ("dtb0", 128, 1), ("dtb1", 64, 1),
    ("invA", 128, 1), ("Dc0", 128, 1), ("Dc1", 64, 1), ("lnb0", 128, 1),
    ("lnb1", 64, 1),
]
CFMAP, CFTOT = _pack(_CF_SHAPES)


def build_program():
    nc = bacc.Bacc("TRN2", target_bir_lowering=False, debug=False)

    xsT0 = nc.declare_dram_parameter("xsT0", [128, L], BF, isOutput=False)
    xsT1 = nc.declare_dram_parameter("xsT1", [64, L], BF, isOutput=False)
    xoT0 = nc.declare_dram_parameter("xoT0", [128, L], BF, isOutput=False)
    xoT1 = nc.declare_dram_parameter("xoT1", [64, L], BF, isOutput=False)
    gq = nc.declare_dram_parameter("gq", [L // 128, 128, T], FP, isOutput=False)
    cpb = nc.declare_dram_parameter("cpb", [128, CBTOT], BF, isOutput=False)
    cpr = nc.declare_dram_parameter("cpr", [128, CRTOT], FPR, isOutput=False)
    cpf = nc.declare_dram_parameter("cpf", [128, CFTOT], FP, isOutput=False)
    yo0 = nc.declare_dram_parameter("yo0", [128, L], FP, isOutput=True)
    yo1 = nc.declare_dram_parameter("yo1", [64, L], FP, isOutput=True)

    with tile.TileContext(nc) as tc:
        with (
            tc.tile_pool(name="const", bufs=1) as cp,
            tc.tile_pool(name="xin", bufs=2) as xp,
            tc.tile_pool(name="dwp", bufs=2) as dwp,
            tc.tile_pool(name="proj", bufs=2) as pj,
            tc.tile_pool(name="route", bufs=2) as rt,
            tc.tile_pool(name="spool", bufs=2) as sp_,
            tc.tile_pool(name="blk", bufs=3) as bk,
            tc.tile_pool(name="hpool", bufs=3) as hp,
            tc.tile_pool(name="ypool", bufs=1) as yp,
            tc.tile_pool(name="rows", bufs=1) as rw,
            tc.tile_pool(name="persist", bufs=1) as pe_,
            tc.tile_pool(name="ps_scan", bufs=2, space="PSUM") as ps_scan,
            tc.tile_pool(name="ps_pre", bufs=1, space="PSUM") as ps_pre,
            tc.tile_pool(name="ps_y", bufs=1, space="PSUM") as ps_y,
        ):
            cbk = cp.tile([128, CBTOT], BF, tag="cpb")
            nc.sync.dma_start(cbk[:], cpb[:])
            crk = cp.tile([128, CRTOT], FPR, tag="cpr")
            nc.sync.dma_start(crk[:], cpr[:])
            cfk = cp.tile([128, CFTOT], FP, tag="cpf")
            nc.sync.dma_start(cfk[:], cpf[:])

            def cb(name):
                off, r, c = CBMAP[name]
                return cbk[0:r, off:off + c]

            def cr(name):
                off, r, c = CRMAP[name]
                return crk[0:r, off:off + c]

            def cf(name):
                off, r, c = CFMAP[name]
                return cfk[0:r, off:off + c]

            def mm512(out, lhsT, rhs, start, stop):
                # ISA caps the moving operand at 512 elements per matmul
                n = rhs.shape[-1]
                for q in range(0, n, 512):
                    e = min(q + 512, n)
                    nc.tensor.matmul(out[:, q:e], lhsT, rhs[:, q:e],
                                     start=start, stop=stop)

            hlast = pe_.tile([P, 24], FP)
            epsc = pe_.tile([128, 1], FP)
            nc.vector.memset(epsc[:], LEPS)

            def emit_preamble(kc):
                """Loads + projections + routing + w-build for chunk kc."""
                ls = kc * LC
                c0 = ls // 128
                C = {}
                xs0 = C["xs0"] = xp.tile([128, LC], BF, tag="xs0", name=f"xs0_{kc}")
                xs1 = C["xs1"] = xp.tile([64, LC], BF, tag="xs1", name=f"xs1_{kc}")
                xo0 = xp.tile([128, LC], BF, tag="xo0", name=f"xo0_{kc}")
                xo1 = xp.tile([64, LC], BF, tag="xo1", name=f"xo1_{kc}")
                gt = xp.tile([128, NT * T], FP, tag="gt", name=f"gt_{kc}")
                nc.sync.dma_start(xs0[:], xsT0[:, ls:ls + LC])
                nc.sync.dma_start(xs1[:], xsT1[:, ls:ls + LC])
                nc.sync.dma_start(xo0[:], xoT0[:, ls:ls + LC])
                nc.sync.dma_start(xo1[:], xoT1[:, ls:ls + LC])
                nc.sync.dma_start(
                    gt[:].rearrange("p (c t) -> p c t", c=NT),
                    gq[c0:c0 + NT].rearrange("c p t -> p c t"))

                # dt -> softplus -> dl ; w = dl*x   (dw = [dl | w])
                dw0 = C["dw0"] = dwp.tile([128, 2 * LC], BF, tag="dw0", name=f"dw0_{kc}")
                dw1 = C["dw1"] = dwp.tile([64, 2 * LC], BF, tag="dw1", name=f"dw1_{kc}")
                dtp0 = ps_pre.tile([128, LC], FP, tag="pre", name=f"dt0{kc}")
                mm512(dtp0[:], cb("Mdt0")[:, 0:128], xs0[:], True, False)
                mm512(dtp0[:], cb("Mdt1")[:, 0:128], xs1[:], False, True)
                # softplus(x) = ln(exp(x) + 1); x = dt + dtb stays < ~3 here
                sp0 = sp_.tile([128, LC], FP, tag="sp0", name=f"sp0_{kc}")
                nc.scalar.activation(sp0[:], dtp0[:], AF.Exp, bias=cf("dtb0"))
                nc.scalar.activation(dw0[:, 0:LC], sp0[:], AF.Ln, bias=1.0)
                dtp1 = ps_pre.tile([64, LC], FP, tag="pre", name=f"dt1{kc}")
                mm512(dtp1[:], cb("Mdt0")[:, 128:DM], xs0[:], True, False)
                mm512(dtp1[:], cb("Mdt1")[:, 128:DM], xs1[:], False, True)
                sp1 = sp_.tile([64, LC], FP, tag="sp1", name=f"sp1_{kc}")
                nc.scalar.activation(sp1[:], dtp1[:], AF.Exp, bias=cf("dtb1"))
                nc.scalar.activation(dw1[:, 0:LC], sp1[:], AF.Ln, bias=1.0)
                nc.gpsimd.tensor_tensor(dw0[:, LC:2 * LC], dw0[:, 0:LC],
                                        xs0[:], OP.mult)
                nc.gpsimd.tensor_tensor(dw1[:, LC:2 * LC], dw1[:, 0:LC],
                                        xs1[:], OP.mult)

                bp = ps_pre.tile([128, LC], FP, tag="pre", name=f"bp{kc}")
                mm512(bp[:], cb("xpB0"), xs0[:], True, False)
                mm512(bp[:], cb("xpB1"), xs1[:], False, True)
                Brep = C["Brep"] = pj.tile([128, LC], BF, tag="Brep", name=f"Brep_{kc}")
                nc.scalar.copy(Brep[:], bp[:])

                # routing of the other stream -> one-hot OT
                zt = ps_pre.tile([128, LC], FP, tag="pre", name=f"zt{kc}")
                mm512(zt[0:H3, :], cb("w1T0"), xo0[:], True, False)
                mm512(zt[0:H3, :], cb("w1T1"), xo1[:], False, True)
                hg = rt.tile([H3, LC], BF, tag="hg", name=f"hg_{kc}")
                nc.scalar.activation(hg[:], zt[0:H3, :], AF.Gelu,
                                     bias=cf("b1c"))
                z2 = ps_scan.tile([128, NT * T], FP, tag="scan", name=f"z2{kc}")
                for i in range(NT):
                    nc.tensor.matmul(z2[:, i * T:(i + 1) * T],
                                     hg[:, i * 128:(i + 1) * 128], cb("w2T"),
                                     start=True, stop=True)
                zg = rt.tile([128, NT * T], FP, tag="zg", name=f"zg_{kc}")
                nc.vector.tensor_tensor(zg[:], z2[:], gt[:], OP.add)
                oh = rt.tile([128, NT * T], FP, tag="oh", name=f"oh_{kc}")
                for i in range(NT):
                    sl = slice(i * T, (i + 1) * T)
                    m8 = rt.tile([128, 8], FP, tag="m8", bufs=3, name=f"m8_{kc}_{i}")
                    nc.vector.max(m8[:], zg[:, sl])
                    nc.vector.tensor_scalar(oh[:, sl], zg[:, sl], m8[:, 0:1],
                                            None, OP.is_equal)
                tp = ps_pre.tile([128, LC], FP, tag="pre", name=f"tp{kc}")
                for i in range(NT):
                    nc.tensor.transpose(tp[0:T, i * 128:(i + 1) * 128],
                                        oh[:, i * T:(i + 1) * T], cf("identF"))
                OT = rt.tile([T, LC], BF, tag="OT", name=f"OT_{kc}")
                nc.scalar.copy(OT[:], tp[0:T, :])

                cpp = ps_pre.tile([128, LC], FP, tag="pre", name=f"cp{kc}")
                mm512(cpp[:], cb("xpC0"), xo0[:], True, False)
                mm512(cpp[:], cb("xpC1"), xo1[:], False, False)
                mm512(cpp[:], cb("PmRep"), OT[:], False, True)
                Crep = C["Crep"] = pj.tile([128, LC], BF, tag="Crep", name=f"Crep_{kc}")
                nc.scalar.copy(Crep[:], cpp[:])
                C["kc"] = kc
                C["hl_pend"] = []
                C["yacc_pend"] = []
                return C


            def emit_yacc(C):
                kc = C["kc"]
                j, sc, ym, rows_, first, last = C["yacc_pend"].pop(0)
                yac = C["yac0"] if j < 16 else C["yac1"]
                mm512(yac[0:rows_, :], sc, ym[:], first, last)
                if j == 15:
                    yD0 = C["yD0"] = yp.tile([128, LC], FPR, tag="yD0",
                                             name=f"yD0_{kc}")
                    nc.vector.scalar_tensor_tensor(
                        yD0[:], C["xs0"][:], cf("Dc0"), C["yac0"][:],
                        OP.mult, OP.add)
                    C["yac1"] = ps_y.tile([64, LC], FP, tag="y",
                                          name=f"y1_{kc}")

            def emit_block(C, j):
                kc = C["kc"]
                if j < 16:
                    dwt = C["dw0"]
                    wa = cb("WdA0")[:, j * P:(j + 1) * P]
                    sc = cb("S80")[:, j * 128:(j + 1) * 128]
                    rows_ = P
                    first, last = j == 0, j == 15
                else:
                    dwt = C["dw1"]
                    wa = cb("WdA1")[:, (j - 16) * P:(j - 15) * P]
                    sc = cb("S81")[:, (j - 16) * 64:(j - 15) * 64]
                    rows_ = 64
                    first, last = j == 16, j == 23
                # deferred hlast copies (Act) — 2 blocks late so Act never
                # stalls waiting for the scan of the current block
                while C["hl_pend"] and C["hl_pend"][0][0] <= j - 2:
                    _, jj, hh = C["hl_pend"].pop(0)
                    nc.scalar.copy(hlast[:, jj:jj + 1], hh[:, LC - 1:LC])
                mmpA = ps_scan.tile([128, LC], FP, tag="scan",
                                    name=f"mmA{kc}_{j}")
                mm512(mmpA[:], wa, dwt[:, 0:LC], True, True)
                dA = bk.tile([P, LC], F16, tag="dA", name=f"dA_{kc}_{j}")
                nc.scalar.activation(dA[:], mmpA[:], AF.Exp)
                mmpB = ps_scan.tile([128, LC], FP, tag="scan",
                                    name=f"mmB{kc}_{j}")
                mm512(mmpB[:], wa, dwt[:, LC:2 * LC], True, True)
                dBu = bk.tile([P, LC], BF, tag="dBu", name=f"dBu_{kc}_{j}")
                if j in DBU_DVE:
                    nc.vector.tensor_tensor(dBu[:], mmpB[:], C["Brep"][:],
                                            OP.mult)
                else:
                    wcp = bk.tile([P, LC], BF, tag="wcp",
                                  name=f"wcp_{kc}_{j}")
                    nc.scalar.copy(wcp[:], mmpB[:])
                    nc.vector.tensor_tensor(dBu[:], wcp[:], C["Brep"][:],
                                            OP.mult)
                h = hp.tile([P, LC], BF, tag="h", name=f"h_{kc}_{j}")
                init = 0.0 if kc == 0 else hlast[:, j:j + 1]
                nc.vector.tensor_tensor_scan(h[:], dA[:], dBu[:], init,
                                             OP.mult, OP.add)
                if kc < NCH - 1:
                    C["hl_pend"].append((j, j, h))
                ym = hp.tile([P, LC], BF, tag="ym", bufs=4,
                             name=f"ym_{kc}_{j}")
                nc.vector.tensor_tensor(ym[:], h[:], C["Crep"][:], OP.mult)
                # yacc matmuls run 2 blocks late so PE never waits on the
                # DVE scan pipeline mid-stream (HAM throttle avoidance)
                C["yacc_pend"].append((j, sc, ym, rows_, first, last))
                while len(C["yacc_pend"]) > 2:
                    emit_yacc(C)

            def emit_ln(C):
                kc = C["kc"]
                ls = kc * LC
                while C["yacc_pend"]:
                    emit_yacc(C)
                while C["hl_pend"]:
                    _, jj, hh = C["hl_pend"].pop(0)
                    nc.scalar.copy(hlast[:, jj:jj + 1], hh[:, LC - 1:LC])
                yD0 = C["yD0"]
                yD1 = yp.tile([64, LC], FPR, tag="yD1", name=f"yD1_{kc}")
                nc.vector.scalar_tensor_tensor(
                    yD1[:], C["xs1"][:], cf("Dc1"), C["yac1"][:],
                    OP.mult, OP.add)
                ysq0 = yp.tile([128, LC], FPR, tag="ysq0", name=f"ysq0_{kc}")
                nc.scalar.activation(ysq0[:], yD0[:].bitcast(FP), AF.Square)
                ysq1 = yp.tile([64, LC], FPR, tag="ysq1", name=f"ysq1_{kc}")
                nc.scalar.activation(ysq1[:], yD1[:].bitcast(FP), AF.Square)

                s1p = ps_scan.tile([128, LC], FP, tag="scan", name=f"s1{kc}")
                mm512(s1p[0:1, :], cr("onc0"), yD0[:], True, False)
                mm512(s1p[0:1, :], cr("onc1"), yD1[:], False, True)
                s2p = ps_scan.tile([128, LC], FP, tag="scan", name=f"s2{kc}")
                mm512(s2p[0:1, :], cr("onc0"), ysq0[:], True, False)
                mm512(s2p[0:1, :], cr("onc1"), ysq1[:], False, True)
                s1row = rw.tile([1, LC], FP, tag="s1row", name=f"s1row_{kc}")
                s2row = rw.tile([1, LC], FP, tag="s2row", name=f"s2row_{kc}")
                nc.scalar.copy(s1row[:], s1p[0:1, :])
                nc.scalar.copy(s2row[:], s2p[0:1, :])
                # stats directly on the [1, LC] row layout
                murow = rw.tile([1, LC], FP, tag="murow", name=f"murow_{kc}")
                nc.vector.tensor_scalar(murow[:], s1row[:], 1.0 / DM, None,
                                        OP.mult)
                msqr = rw.tile([1, LC], FP, tag="msqr", name=f"msqr_{kc}")
                nc.scalar.activation(msqr[:], murow[:], AF.Square)
                varr = rw.tile([1, LC], FP, tag="varr", name=f"varr_{kc}")
                nc.vector.scalar_tensor_tensor(
                    varr[:], s2row[:], 1.0 / DM, msqr[:],
                    OP.mult, OP.subtract)
                irow = rw.tile([1, LC], FPR, tag="irow", name=f"irow_{kc}")
                nc.scalar.activation(irow[:], varr[:], AF.Abs_reciprocal_sqrt,
                                     bias=epsc[0:1, :])
                mirow = rw.tile([1, LC], FPR, tag="mirow", name=f"mirow_{kc}")
                with nc.allow_low_precision(reason="fp32r rows for broadcast"):
                    nc.vector.tensor_tensor(mirow[:], murow[:],
                                            irow[:].bitcast(FP), OP.mult)

                # broadcast g*inv and g*mu*inv via k=1 fp32 matmuls
                ib0 = ps_scan.tile([128, LC], FP, tag="scan", name=f"ib0{kc}")
                mi0 = ps_scan.tile([128, LC], FP, tag="scan", name=f"mi0{kc}")
                mm512(ib0[:], cr("lngr0"), irow[:], True, True)
                mm512(mi0[:], cr("lngr0"), mirow[:], True, True)
                yo0t = yp.tile([128, LC], FP, tag="yo0t", name=f"yo0t_{kc}")
                nc.vector.tensor_tensor(yo0t[:], yD0[:].bitcast(FP), ib0[:],
                                        OP.mult)
                nc.vector.scalar_tensor_tensor(
                    yo0t[:], yo0t[:], cf("lnb0"), mi0[:], OP.add, OP.subtract)
                nc.sync.dma_start(yo0[:, ls:ls + LC], yo0t[:])

                ib1 = ps_scan.tile([64, LC], FP, tag="scan", name=f"ib1{kc}")
                mi1 = ps_scan.tile([64, LC], FP, tag="scan", name=f"mi1{kc}")
                mm512(ib1[:], cr("lngr1"), irow[:], True, True)
                mm512(mi1[:], cr("lngr1"), mirow[:], True, True)
                yo1t = yp.tile([64, LC], FP, tag="yo1t", name=f"yo1t_{kc}")
                nc.vector.tensor_tensor(yo1t[:], yD1[:].bitcast(FP), ib1[:],
                                        OP.mult)
                nc.vector.scalar_tensor_tensor(
                    yo1t[:], yo1t[:], cf("lnb1"), mi1[:], OP.add, OP.subtract)
                nc.sync.dma_start(yo1[:, ls:ls + LC], yo1t[:])

            # ---- software-pipelined chunk loop ----
            Ccur = emit_preamble(0)
            Ccur["yac0"] = ps_y.tile([128, LC], FP, tag="y", name="y0_0")
            for kc in range(NCH):
                for j in range(2):
                    emit_block(Ccur, j)
                Cnext = emit_preamble(kc + 1) if kc + 1 < NCH else None
                for j in range(2, 24):
                    emit_block(Ccur, j)
                if Cnext is not None:
                    Cnext["yac0"] = ps_y.tile([128, LC], FP, tag="y",
                                              name=f"y0_{kc + 1}")
                emit_ln(Ccur)
                Ccur = Cnext

    nc.compile()
    return nc


_PROG = None


def _get_prog():
    global _PROG
    if _PROG is None:
        _PROG = build_program()
    return _PROG


def _make_in_maps(inputs):
    f32 = lambda a: np.ascontiguousarray(np.asarray(a, dtype=np.float32))
    bf16 = lambda a: np.ascontiguousarray(
        np.asarray(np.asarray(a, dtype=np.float32), dtype=ml_dtypes.bfloat16))
    x = {0: f32(inputs["x_rgb"]), 1: f32(inputs["x_e"])}
    u = {0: f32(inputs["u_rgb"]), 1: f32(inputs["u_e"])}
    rw1 = {0: f32(inputs["route_rgb_w1"]), 1: f32(inputs["route_e_w1"])}
    rb1 = {0: f32(inputs["route_rgb_b1"]), 1: f32(inputs["route_e_b1"])}
    rw2 = {0: f32(inputs["route_rgb_w2"]), 1: f32(inputs["route_e_w2"])}
    rb2 = {0: f32(inputs["route_rgb_b2"]), 1: f32(inputs["route_e_b2"])}
    emb = {0: f32(inputs["emb_rgb"]), 1: f32(inputs["emb_e"])}
    tok = {0: f32(inputs["token_rgb_w"]), 1: f32(inputs["token_e_w"])}
    xproj = {0: f32(inputs["xproj_rgb"]), 1: f32(inputs["xproj_e"])}
    dtw = {0: f32(inputs["dtw_rgb"]), 1: f32(inputs["dtw_e"])}
    dtb = {0: f32(inputs["dtb_rgb"]), 1: f32(inputs["dtb_e"])}
    Alog = {0: f32(inputs["Alog_rgb"]), 1: f32(inputs["Alog_e"])}
    Dsk = {0: f32(inputs["D_rgb"]), 1: f32(inputs["D_e"])}
    lng = {0: f32(inputs["ln1_g"]), 1: f32(inputs["ln2_g"])}
    lnb = {0: f32(inputs["ln1_b"]), 1: f32(inputs["ln2_b"])}

    nmap = np.arange(P) % 16   # p -> n
    dmap = np.arange(P) // 16  # p -> d8

    in_maps = []
    for c in range(8):
        b, s = divmod(c, 2)
        o = 1 - s
        xsT = x[s][b].T.copy()          # [192, L]
        xoT = x[o][b].T.copy()
        A = -np.exp(Alog[s])            # [DM, N]
        assert np.allclose(A, A[0:1, :], atol=0), "A must be d-independent"
        Arow = A[0]                     # [N]
        WdA0 = np.zeros((16, 128, P), np.float32)
        for j in range(16):
            WdA0[j, 8 * j + dmap, np.arange(P)] = Arow[nmap]
        WdA1 = np.zeros((8, 64, P), np.float32)
        for j in range(8):
            WdA1[j, 8 * j + dmap, np.arange(P)] = Arow[nmap]
        S80 = np.zeros((16, P, 128), np.float32)
        for j in range(16):
            S80[j, np.arange(P), 8 * j + dmap] = 1.0
        S81 = np.zeros((8, P, 64), np.float32)
        for j in range(8):
            S81[j, np.arange(P), 8 * j + dmap] = 1.0
        Pm = emb[o] @ tok[o]            # [T, N]
        PmRep = np.ascontiguousarray(Pm[:, nmap])                 # [T, P]
        CrepT = np.ascontiguousarray(xproj[o][R + N:R + 2 * N][nmap].T)
        BrepT = np.ascontiguousarray((xproj[s][R:R + N][nmap]
                                      * (1.0 / Arow[nmap])[:, None]).T)
        Mdt = (dtw[s] @ xproj[s][:R]).T.copy()                    # [DM, DM]

        cb_consts = {
            "w1T0": rw1[o].T[:128], "w1T1": rw1[o].T[128:], "w2T": rw2[o].T,
            "PmRep": PmRep, "xpC0": CrepT[:128], "xpC1": CrepT[128:],
            "xpB0": BrepT[:128], "xpB1": BrepT[128:],
            "Mdt0": Mdt[:128], "Mdt1": Mdt[128:],
            "S80": np.transpose(S80, (1, 0, 2)).reshape(P, 16 * 128),
            "S81": np.transpose(S81, (1, 0, 2)).reshape(P, 8 * 64),
            "WdA0": np.transpose(WdA0, (1, 0, 2)).reshape(128, 16 * P),
            "WdA1": np.transpose(WdA1, (1, 0, 2)).reshape(64, 8 * P),
        }
        cpb_arr = np.zeros((128, CBTOT), np.float32)
        for nm, (off, r, ccols) in CBMAP.items():
            a = np.asarray(cb_consts[nm], np.float32)
            assert a.shape == (r, ccols), (nm, a.shape)
            cpb_arr[:r, off:off + ccols] = a

        cr_consts = {
            "onc0": np.ones((128, 1), np.float32),
            "onc1": np.ones((64, 1), np.float32),
            "lngr0": lng[s][None, :128], "lngr1": lng[s][None, 128:],
        }
        cpr_arr = np.zeros((128, CRTOT), np.float32)
        for nm, (off, r, ccols) in CRMAP.items():
            a = np.asarray(cr_consts[nm], np.float32)
            assert a.shape == (r, ccols), (nm, a.shape)
            cpr_arr[:r, off:off + ccols] = a

        cf_consts = {
            "identF": np.eye(128, dtype=np.float32),
            "b1c": rb1[o][:, None], "dtb0": dtb[s][:128, None],
            "dtb1": dtb[s][128:, None],
            "invA": (1.0 / Arow[nmap])[:, None],
            "Dc0": Dsk[s][:128, None], "Dc1": Dsk[s][128:, None],
            "lnb0": lnb[s][:128, None], "lnb1": lnb[s][128:, None],
        }
        cpf_arr = np.zeros((128, CFTOT), np.float32)
        for nm, (off, r, ccols) in CFMAP.items():
            a = np.asarray(cf_consts[nm], np.float32)
            assert a.shape == (r, ccols), (nm, a.shape)
            cpf_arr[:r, off:off + ccols] = a

        gqa = (-np.log(-np.log(u[o][b])) + rb2[o][None, :]).astype(np.float32)
        m = {
            "xsT0": bf16(xsT[:128]), "xsT1": bf16(xsT[128:]),
            "xoT0": bf16(xoT[:128]), "xoT1": bf16(xoT[128:]),
            "gq": gqa.reshape(L // 128, 128, T).copy(),
            "cpb": np.ascontiguousarray(cpb_arr.astype(ml_dtypes.bfloat16)),
            "cpr": cpr_arr,
            "cpf": cpf_arr,
        }
        in_maps.append(m)
    return in_maps


def run(inputs, trace=False):
    nc = _get_prog()
    in_maps = _make_in_maps(inputs)
    res = run_bass_kernel_spmd(nc, in_maps, list(range(8)), trace=trace)
    out = np.zeros((2, B, DM, L), np.float32)
    for c in range(8):
        b, s = divmod(c, 2)
        out[s, b, :128] = res.results[c]["yo0"]
        out[s, b, 128:] = res.results[c]["yo1"]
    return out, res


def kernel(**inputs):
    out, _ = run(inputs, trace=False)
    return out

